# revision 7
# baseline (speedup 1.0000x reference)
"""Trainium2 Bass kernel for the patch-CNN problem (v2).

Math: x [2048,2,25,25] --bicubic-up--> [2048,2,50,50] --5x5-grid of 10x10
patches--> per-patch 3-layer CNN (two weight branches phi/p, 25 patch-specific
weight sets each) --> reassemble [2048,2,50,50].

Every stage (upsample, conv1, conv2, conv3) is a banded matrix acting on
activation vectors laid out [(spatial, channel), batch] in SBUF; batch is the
matmul moving dimension. Band matrices are built on the host from the input
weights and cut into 128x128 blocks; only structurally-nonzero blocks execute.

v2 over v1:
  - bf16 matmul operands (PSUM accumulation stays fp32); rel err ~5e-3
  - core <-> (branch, batch-quarter): each core runs all 25 patches of one
    branch on a 512-image quarter => 25 identical slots/core, perfect load
    balance (5050 matmuls/core vs 5252 on the worst v1 core)
  - per-slot weight array shipped as ONE 6.6MB bf16 DMA, double-buffered =>
    PE never waits on weights after the first slot
  - xs (the 9 input rows a patch-row needs) loaded once per patch-row i and
    shared by its 5 patches: 5 xs loads per core instead of 25
"""

import numpy as np
from contextlib import ExitStack

import ml_dtypes

import concourse.bass as bass
import concourse.mybir as mybir
import concourse.tile as tile
from concourse import bacc
from concourse.bass_utils import run_bass_kernel_spmd

# ---------------------------------------------------------------- constants
B = 2048
C_IN = 2
W1C = 16
W2C = 32
KK = 5
NP = 25
L = 10
OUT = 50
N_CORES = 8
BQ = 512               # images per slot (batch quarter; also the moving tile)

H_WIN = 9              # input rows needed per patch row (bicubic support)
XS_ROWS = H_WIN * 25 * C_IN   # 450 real rows
XS_PAD = 512                  # padded to 4 k-tiles
XUP_ROWS = 200                # (y'',x'',ci) per patch
H1_ROWS = 100 * W1C           # 1600
H2_ROWS = 100 * W2C           # 3200
OUT_ROWS = 100

f32 = mybir.dt.float32
bf16 = mybir.dt.bfloat16
BF16 = ml_dtypes.bfloat16


def _h0(i):
    return min(max(5 * i - 2, 0), 25 - H_WIN)


def bicubic_matrix(out_size, in_size):
    """Row-stochastic resize matrix identical to jax.image.resize bicubic."""
    scale = out_size / in_size
    u = (np.arange(out_size) + 0.5) / scale - 0.5
    s = np.abs(u[:, None] - np.arange(in_size)[None, :])
    A = -0.5
    w = np.where(
        s <= 1,
        (A + 2) * s**3 - (A + 3) * s**2 + 1,
        np.where(s < 2, A * s**3 - 5 * A * s**2 + 8 * A * s - 4 * A, 0.0),
    )
    w = w / w.sum(axis=1, keepdims=True)
    return w.astype(np.float64)


_R = bicubic_matrix(OUT, 25)           # [50, 25]
_R_NZ = np.abs(_R) > 1e-300


# ------------------------------------------------------- structural schedule
def _grids(krows, mrows, kc, mc, kh, mw):
    kl = kc * 128 + np.arange(kh)
    ml = mc * 128 + np.arange(mw)
    return kl[:, None], ml[None, :]


def _conv_struct(krows, mrows, kdiv, mdiv):
    """Sparsity pattern for T[(s'*kdiv+ci),(s*mdiv+co)] with 5x5 taps."""
    k = np.arange(krows)
    m = np.arange(mrows)
    sp, s = k // kdiv, m // mdiv
    yp, xp = sp // 10, sp % 10
    y, x = s // 10, s % 10
    return (np.abs(yp[:, None] - y[None, :]) <= 2) & (
        np.abs(xp[:, None] - x[None, :]) <= 2
    )


def _up_struct():
    """Union-over-patches sparsity of the upsample matrix [XS_PAD, XUP_ROWS]."""
    nz = np.zeros((XS_PAD, XUP_ROWS), dtype=bool)
    k = np.arange(XS_ROWS)
    hl, wv, ci = k // 50, (k % 50) // 2, k % 2
    m = np.arange(XUP_ROWS)
    spp, cip = m // 2, m % 2
    ypp, xpp = spp // 10, spp % 10
    for i in range(5):
        for j in range(5):
            lv = _R_NZ[10 * i + ypp, :][:, _h0(i) + hl]  # [M, K]
            rv = _R_NZ[10 * j + xpp, :][:, wv]
            nzij = (lv & rv).T & (ci[:, None] == cip[None, :])
            nz[:XS_ROWS] |= nzij
    return nz


def _blocks_of(nzmask, krows, mrows):
    """For each 128-col chunk, the list of 128-row chunks with any nonzero."""
    nkc = (krows + 127) // 128
    nmc = (mrows + 127) // 128
    out = []
    for mc in range(nmc):
        mw = min(128, mrows - mc * 128)
        kcs = []
        for kc in range(nkc):
            kh = min(128, krows - kc * 128)
            if nzmask[kc * 128 : kc * 128 + kh, mc * 128 : mc * 128 + mw].any():
                kcs.append((kc, kh))
        out.append((mw, kcs))
    return out


_SCHED = {
    "up": _blocks_of(_up_struct(), XS_PAD, XUP_ROWS),
    "c1": _blocks_of(_conv_struct(XUP_ROWS, H1_ROWS, 2, W1C), XUP_ROWS, H1_ROWS),
    "c2": _blocks_of(_conv_struct(H1_ROWS, H2_ROWS, W1C, W2C), H1_ROWS, H2_ROWS),
    "c3": _blocks_of(_conv_struct(H2_ROWS, OUT_ROWS, W2C, 1), H2_ROWS, OUT_ROWS),
}

# column layout of the per-unit weight-block array
_GROUPS = {}   # (layer, mc) -> (col_off, n_blocks)
_col = 0
for _lay in ("up", "c1", "c2", "c3"):
    for _mc, (_mw, _kcs) in enumerate(_SCHED[_lay]):
        _GROUPS[(_lay, _mc)] = (_col, len(_kcs))
        _col += len(_kcs) * 128
TOTCOLS = _col

_N_MC = {lay: len(_SCHED[lay]) for lay in _SCHED}
_BIAS_COLS = _N_MC["c1"] + _N_MC["c2"] + 1   # conv1, conv2, conv3 bias columns
_BC1, _BC2, _BC3 = 0, _N_MC["c1"], _N_MC["c1"] + _N_MC["c2"]

# per-patch up-stage schedule: the union layout has 8 blocks but a given
# patch only needs ~5.8 — skip its structurally-zero blocks (wb layout keeps
# the union column positions, so idx offsets are unchanged)
def _up_sched_for_patch(p):
    i, j = p // 5, p % 5
    k = np.arange(XS_ROWS)
    hl, wv, ci = k // 50, (k % 50) // 2, k % 2
    m = np.arange(XUP_ROWS)
    spp, cip = m // 2, m % 2
    ypp, xpp = spp // 10, spp % 10
    lv = _R_NZ[10 * i + ypp, :][:, _h0(i) + hl]
    rv = _R_NZ[10 * j + xpp, :][:, wv]
    nz = np.zeros((XS_PAD, XUP_ROWS), bool)
    nz[:XS_ROWS] = (lv & rv).T & (ci[:, None] == cip[None, :])
    out = []
    for mc, (mw, kcs) in enumerate(_SCHED["up"]):
        keep = []
        for idx, (kc, kh) in enumerate(kcs):
            if nz[kc * 128 : kc * 128 + kh, mc * 128 : mc * 128 + mw].any():
                keep.append((idx, kc, kh))
        out.append((mw, keep))
    return out


_UP_SCHED_P = [_up_sched_for_patch(p) for p in range(NP)]

# weight-DMA sub-transfers: [up+c1 | c2 in thirds | c3] so early layers
# of a slot can start before the whole 6.6MB slot array lands
_C2_OFF = _GROUPS[("c2", 0)][0]
_C3_OFF = _GROUPS[("c3", 0)][0]
_C2_CUT1 = _GROUPS[("c2", 8)][0]
_C2_CUT2 = _GROUPS[("c2", 17)][0]
_WSPLITS = [(0, _C2_OFF), (_C2_OFF, _C2_CUT1), (_C2_CUT1, _C2_CUT2),
            (_C2_CUT2, _C3_OFF), (_C3_OFF, TOTCOLS)]

# 50 weight units ordered (branch, patch); core c runs units br=c//4
N_UNITS = 2 * NP


# ------------------------------------------------- host weight-block builder
def _conv_gather(krows, mrows, kdiv, mdiv, nci, kc, mc, kh, mw):
    """Gather indices+mask for a conv block: widx into w[co,ci,ky,kx] flat."""
    kl, ml = _grids(krows, mrows, kc, mc, kh, mw)
    sp, ci = kl // kdiv, kl % kdiv
    s, co = ml // mdiv, ml % mdiv
    yp, xp = sp // 10, sp % 10
    y, x = s // 10, s % 10
    ky = yp - y + 2
    kx = xp - x + 2
    valid = (ky >= 0) & (ky < 5) & (kx >= 0) & (kx < 5)
    ky = np.clip(ky, 0, 4)
    kx = np.clip(kx, 0, 4)
    widx = ((co * nci + ci) * 5 + ky) * 5 + kx
    return widx, valid


_CONV_GATHER_CACHE = {}


def _conv_gather_cached(lay, kc, mc, kh, mw):
    key = (lay, kc, mc)
    if key not in _CONV_GATHER_CACHE:
        if lay == "c1":
            g = _conv_gather(XUP_ROWS, H1_ROWS, 2, W1C, C_IN, kc, mc, kh, mw)
        elif lay == "c2":
            g = _conv_gather(H1_ROWS, H2_ROWS, W1C, W2C, W1C, kc, mc, kh, mw)
        else:
            g = _conv_gather(H2_ROWS, OUT_ROWS, W2C, 1, W2C, kc, mc, kh, mw)
        _CONV_GATHER_CACHE[key] = g
    return _CONV_GATHER_CACHE[key]


def _build_wblocks(w1f, w2f, w3f, ij):
    """Build [n_units, 128, TOTCOLS] float32 weight-block array.

    w1f [nu, 16*2*25], w2f [nu, 32*16*25], w3f [nu, 32*25]; ij [nu, 2]
    """
    nu = w1f.shape[0]
    wall = np.zeros((nu, 128, TOTCOLS), dtype=np.float32)

    # upsample blocks: value = R[10i+y'', h0(i)+h_loc] * R[10j+x'', w] * (ci==ci')
    k = np.arange(XS_PAD)
    hl, wv, ci = k // 50, (k % 50) // 2, k % 2
    hl = np.where(k < XS_ROWS, hl, 0)
    m = np.arange(XUP_ROWS)
    spp, cip = m // 2, m % 2
    ypp, xpp = spp // 10, spp % 10
    iu = ij[:, 0]
    ju = ij[:, 1]
    h0u = np.minimum(np.maximum(5 * iu - 2, 0), 25 - H_WIN)
    # left[u, m, k] = R[10*iu+ypp[m], h0u+hl[k]]  (zero beyond real rows)
    left = _R[(10 * iu[:, None, None] + ypp[None, :, None]),
              (h0u[:, None, None] + hl[None, None, :])]
    left *= (np.arange(XS_PAD)[None, None, :] < XS_ROWS)
    right = _R[(10 * ju[:, None, None] + xpp[None, :, None]),
               wv[None, None, :]]
    same = (ci[None, :] == cip[:, None])[None, :, :]          # [1, M, K]
    upmat = (left * right * same).transpose(0, 2, 1).astype(np.float32)  # [nu,K,M]
    for mc, (mw, kcs) in enumerate(_SCHED["up"]):
        off, _n = _GROUPS[("up", mc)]
        for idx, (kc, kh) in enumerate(kcs):
            wall[:, :kh, off + idx * 128 : off + idx * 128 + mw] = upmat[
                :, kc * 128 : kc * 128 + kh, mc * 128 : mc * 128 + mw
            ]

    for lay, wf in (("c1", w1f), ("c2", w2f), ("c3", w3f)):
        for mc, (mw, kcs) in enumerate(_SCHED[lay]):
            off, _n = _GROUPS[(lay, mc)]
            for idx, (kc, kh) in enumerate(kcs):
                widx, valid = _conv_gather_cached(lay, kc, mc, kh, mw)
                blk = wf[:, widx.reshape(-1)].reshape(nu, kh, mw)
                blk = blk * valid[None, :, :]
                wall[:, :kh, off + idx * 128 : off + idx * 128 + mw] = blk
    return wall


def _relu_bias(eng, out_ap, ps_ap, bias_ap):
    """out = max(ps + bias, 0) — ACT engine via activation, DVE/Pool via
    fused tensor_scalar."""
    import concourse.mybir as _mb
    if hasattr(eng, "activation"):
        eng.activation(out_ap, ps_ap, _mb.ActivationFunctionType.Relu,
                       bias=bias_ap)
    else:
        eng.tensor_scalar(out_ap, ps_ap, bias_ap, 0.0,
                          _mb.AluOpType.add, _mb.AluOpType.max)


# --------------------------------------------------------- device program
_NC_CACHE = None


def _build_nc():
    nc = bacc.Bacc("TRN2", target_bir_lowering=False, debug=False,
                   num_devices=N_CORES)
    xs_d = nc.dram_tensor("xs", [5, XS_PAD, BQ], bf16, kind="ExternalInput").ap()
    wb_d = nc.dram_tensor("wb", [NP, 128, TOTCOLS], bf16,
                          kind="ExternalInput").ap()
    bias_d = nc.dram_tensor("bias", [NP, 128, _BIAS_COLS], f32,
                            kind="ExternalInput").ap()
    y_d = nc.dram_tensor("y", [NP, OUT_ROWS, BQ], f32, kind="ExternalOutput").ap()

    AF = mybir.ActivationFunctionType
    mw3, kcs3 = _SCHED["c3"][0]
    kh3_map = dict(kcs3)
    off3, _n3 = _GROUPS[("c3", 0)]
    n_c2 = _N_MC["c2"]

    with tile.TileContext(nc) as tc, ExitStack() as ctx:
        xs_pool = ctx.enter_context(tc.tile_pool(name="xs", bufs=2))
        xup_pool = ctx.enter_context(tc.tile_pool(name="xup", bufs=2))
        h1_pool = ctx.enter_context(tc.tile_pool(name="h1", bufs=1))
        h2_pool = ctx.enter_context(tc.tile_pool(name="h2", bufs=6))
        w_pool = ctx.enter_context(tc.tile_pool(name="w", bufs=2))
        bias_pool = ctx.enter_context(tc.tile_pool(name="bias", bufs=2))
        out_pool = ctx.enter_context(tc.tile_pool(name="out", bufs=2))
        ps_pool = ctx.enter_context(tc.tile_pool(name="ps", bufs=6, space="PSUM"))
        ps3_pool = ctx.enter_context(tc.tile_pool(name="ps3", bufs=2, space="PSUM"))

        for i in range(5):
            # xs for this patch-row, shared by its 5 patches
            xs_t = []
            for t in range(XS_PAD // 128):
                xt = xs_pool.tile([128, BQ], bf16, tag=f"xs{t}")
                nc.sync.dma_start(out=xt[:], in_=xs_d[i, 128 * t : 128 * (t + 1), :])
                xs_t.append(xt)

            for j in range(5):
                u = 5 * i + j
                # whole-slot weight array, one transfer, double-buffered
                wt = w_pool.tile([128, TOTCOLS], bf16, tag="w")
                for c0, c1 in _WSPLITS:
                    nc.gpsimd.dma_start(out=wt[:, c0:c1], in_=wb_d[u, :, c0:c1])
                bias_t = bias_pool.tile([128, _BIAS_COLS], f32, tag="bias")
                nc.sync.dma_start(out=bias_t[:], in_=bias_d[u])

                def mm_group(lay, mc, mw, kcs, src_tiles, ps):
                    off, _n = _GROUPS[(lay, mc)]
                    for idx, (kc, kh) in enumerate(kcs):
                        nc.tensor.matmul(
                            ps[:mw, :],
                            wt[:kh, off + idx * 128 : off + idx * 128 + mw],
                            src_tiles[kc][:kh, :],
                            start=(idx == 0),
                            stop=(idx == len(kcs) - 1),
                        )

                # ---- upsample -> xup tiles (patch-specific block subset)
                xup_t = []
                for mc, (mw, keep) in enumerate(_UP_SCHED_P[u]):
                    off, _n = _GROUPS[("up", mc)]
                    ps = ps_pool.tile([128, BQ], f32, tag="ps")
                    for n_i, (idx, kc, kh) in enumerate(keep):
                        nc.tensor.matmul(
                            ps[:mw, :],
                            wt[:kh, off + idx * 128 : off + idx * 128 + mw],
                            xs_t[kc][:kh, :],
                            start=(n_i == 0),
                            stop=(n_i == len(keep) - 1),
                        )
                    xt2 = xup_pool.tile([128, BQ], bf16, tag=f"xup{mc}")
                    nc.vector.tensor_copy(xt2[:mw, :], ps[:mw, :])
                    xup_t.append(xt2)

                # ---- conv1 -> h1 tiles (relu + bias)
                h1_t = []
                for mc, (mw, kcs) in enumerate(_SCHED["c1"]):
                    ps = ps_pool.tile([128, BQ], f32, tag="ps")
                    mm_group("c1", mc, mw, kcs, xup_t, ps)
                    ht = h1_pool.tile([128, BQ], bf16, tag=f"h1_{mc}")
                    _relu_bias(nc.scalar if mc % 2 == 0 else nc.vector,
                               ht[:mw, :], ps[:mw, :],
                               bias_t[:mw, _BC1 + mc : _BC1 + mc + 1])
                    h1_t.append(ht)

                # ---- conv2 + interleaved conv3 accumulation
                # conv3's matmul for group mc is emitted one c2-group late so
                # the PE never waits on the h2 activation latency
                ps3 = ps3_pool.tile([128, BQ], f32, tag="ps3")

                def c3_mm(mc, h2t):
                    nc.tensor.matmul(
                        ps3[:mw3, :],
                        wt[: kh3_map[mc], off3 + mc * 128 : off3 + mc * 128 + mw3],
                        h2t[: kh3_map[mc], :],
                        start=(mc == 0),
                        stop=(mc == n_c2 - 1),
                        skip_group_check=True,
                    )

                pending = None   # (mc, h2t) awaiting its conv3 matmul
                for mc, (mw, kcs) in enumerate(_SCHED["c2"]):
                    ps = ps_pool.tile([128, BQ], f32, tag="ps")
                    mm_group("c2", mc, mw, kcs, h1_t, ps)
                    if pending is not None:
                        c3_mm(*pending)
                    h2t = h2_pool.tile([128, BQ], bf16, tag="h2")
                    _relu_bias(nc.scalar if mc % 2 == 0 else nc.vector,
                               h2t[:mw, :], ps[:mw, :],
                               bias_t[:mw, _BC2 + mc : _BC2 + mc + 1])
                    pending = (mc, h2t)
                c3_mm(*pending)

                # ---- conv3 bias + store
                ot = out_pool.tile([128, BQ], f32, tag="out")
                nc.scalar.activation(
                    ot[:mw3, :], ps3[:mw3, :], AF.Identity,
                    bias=bias_t[:mw3, _BC3 : _BC3 + 1],
                )
                nc.sync.dma_start(out=y_d[u], in_=ot[:OUT_ROWS, :])

    nc.compile()
    return nc


# ----------------------------------------------------------- host pipeline
LAST_RESULTS = None


def _unit_weight_flats(inputs):
    """Per-unit flattened conv weights/biases, unit order = (branch, patch)."""
    w1 = np.stack([np.asarray(inputs["phi_w1"]), np.asarray(inputs["p_w1"])], 0)
    w2 = np.stack([np.asarray(inputs["phi_w2"]), np.asarray(inputs["p_w2"])], 0)
    w3 = np.stack([np.asarray(inputs["phi_w3"]), np.asarray(inputs["p_w3"])], 0)
    b1 = np.stack([np.asarray(inputs["phi_b1"]), np.asarray(inputs["p_b1"])], 0)
    b2 = np.stack([np.asarray(inputs["phi_b2"]), np.asarray(inputs["p_b2"])], 0)
    b3 = np.stack([np.asarray(inputs["phi_b3"]), np.asarray(inputs["p_b3"])], 0)
    # [br, p, ...] -> [br*25+p, flat]
    w1f = w1.reshape(N_UNITS, -1).astype(np.float32)
    w2f = w2.reshape(N_UNITS, -1).astype(np.float32)
    w3f = w3.reshape(N_UNITS, -1).astype(np.float32)
    b1u = b1.reshape(N_UNITS, W1C).astype(np.float32)
    b2u = b2.reshape(N_UNITS, W2C).astype(np.float32)
    b3u = b3.reshape(N_UNITS).astype(np.float32)
    p = np.tile(np.arange(NP), 2)
    ij = np.stack([p // 5, p % 5], 1)
    return w1f, w2f, w3f, b1u, b2u, b3u, ij


def _build_host_inputs(inputs):
    x = np.asarray(inputs["x"], dtype=np.float32)
    w1f, w2f, w3f, b1u, b2u, b3u, ij = _unit_weight_flats(inputs)
    wall = _build_wblocks(w1f, w2f, w3f, ij).astype(BF16)  # [50, 128, TOTCOLS]

    # biases [50, 128, BIAS_COLS]
    ball = np.zeros((N_UNITS, 128, _BIAS_COLS), np.float32)
    for mc, (mw, _k) in enumerate(_SCHED["c1"]):
        ml = mc * 128 + np.arange(mw)
        ball[:, :mw, _BC1 + mc] = b1u[:, ml % W1C]
    for mc, (mw, _k) in enumerate(_SCHED["c2"]):
        ml = mc * 128 + np.arange(mw)
        ball[:, :mw, _BC2 + mc] = b2u[:, ml % W2C]
    ball[:, :OUT_ROWS, _BC3] = b3u[:, None]

    # x slices per patch row i: [5, XS_PAD, B]
    xt = np.ascontiguousarray(x.transpose(2, 3, 1, 0))  # [h, w, ci, b]
    xs_all = np.zeros((5, XS_PAD, B), np.float32)
    for i in range(5):
        h0 = _h0(i)
        xs_all[i, :XS_ROWS] = xt[h0 : h0 + H_WIN].reshape(XS_ROWS, B)
    xs_all = xs_all.astype(BF16)

    in_maps = []
    for c in range(N_CORES):
        br, q = c // 4, c % 4
        in_maps.append({
            "xs": np.ascontiguousarray(xs_all[:, :, q * BQ : (q + 1) * BQ]),
            "wb": wall[br * NP : (br + 1) * NP],
            "bias": ball[br * NP : (br + 1) * NP],
        })
    return in_maps


def _assemble(results):
    out = np.zeros((B, 2, OUT, OUT), np.float32)
    for c in range(N_CORES):
        br, q = c // 4, c % 4
        y = results[c]["y"]  # [25, 100, BQ]
        for p in range(NP):
            i, j = p // 5, p % 5
            blk = y[p].reshape(L, L, BQ).transpose(2, 0, 1)
            out[q * BQ : (q + 1) * BQ, br,
                10 * i : 10 * i + L, 10 * j : 10 * j + L] = blk
    return out


def kernel(**inputs):
    global _NC_CACHE, LAST_RESULTS
    in_maps = _build_host_inputs(inputs)
    if _NC_CACHE is None:
        _NC_CACHE = _build_nc()
    res = run_bass_kernel_spmd(_NC_CACHE, in_maps, list(range(N_CORES)))
    LAST_RESULTS = res
    return _assemble(res.results)


# ------------------------------------------------- numpy emulation (debug)
def emulate(**inputs):
    """Pure-numpy emulation of the exact device dataflow (for debugging)."""
    in_maps = _build_host_inputs(inputs)
    results = []
    for c in range(N_CORES):
        m = in_maps[c]
        wbf = np.asarray(m["wb"], dtype=np.float32)
        xsf = np.asarray(m["xs"], dtype=np.float32)
        y = np.zeros((NP, OUT_ROWS, BQ), np.float32)
        for u in range(NP):
            acts = {"xs": xsf[u // 5]}
            srcs = {"up": "xs", "c1": "up", "c2": "c1", "c3": "c2"}
            rows = {"up": XUP_ROWS, "c1": H1_ROWS, "c2": H2_ROWS, "c3": OUT_ROWS}
            for lay in ("up", "c1", "c2", "c3"):
                src = acts[srcs[lay]]
                dst = np.zeros((rows[lay], BQ), np.float32)
                for mc, (mw, kcs) in enumerate(_SCHED[lay]):
                    off, _n = _GROUPS[(lay, mc)]
                    pacc = np.zeros((mw, BQ), np.float32)
                    for idx, (kc, kh) in enumerate(kcs):
                        blk = wbf[u][:kh, off + idx * 128 : off + idx * 128 + mw]
                        pacc += blk.T @ src[kc * 128 : kc * 128 + kh]
                    dst[mc * 128 : mc * 128 + mw] = pacc
                if lay == "c1":
                    dst = np.maximum(
                        dst + m["bias"][u][
                            np.arange(H1_ROWS) % 128, _BC1 + np.arange(H1_ROWS) // 128
                        ][:, None], 0)
                elif lay == "c2":
                    dst = np.maximum(
                        dst + m["bias"][u][
                            np.arange(H2_ROWS) % 128, _BC2 + np.arange(H2_ROWS) // 128
                        ][:, None], 0)
                elif lay == "c3":
                    dst = dst + m["bias"][u][:OUT_ROWS, _BC3][:, None]
                    acts[lay] = dst
                    continue
                # device stores inter-layer activations in bf16
                acts[lay] = q_bf16(dst)
            y[u] = acts["c3"]
        results.append({"y": y})
    return _assemble(results)


def q_bf16(a):
    return a.astype(BF16).astype(np.float32)


# revision 8
# speedup vs baseline: 1.0079x; 1.0079x over previous
"""Trainium2 Bass kernel for the patch-CNN problem (v2).

Math: x [2048,2,25,25] --bicubic-up--> [2048,2,50,50] --5x5-grid of 10x10
patches--> per-patch 3-layer CNN (two weight branches phi/p, 25 patch-specific
weight sets each) --> reassemble [2048,2,50,50].

Every stage (upsample, conv1, conv2, conv3) is a banded matrix acting on
activation vectors laid out [(spatial, channel), batch] in SBUF; batch is the
matmul moving dimension. Band matrices are built on the host from the input
weights and cut into 128x128 blocks; only structurally-nonzero blocks execute.

v2 over v1:
  - bf16 matmul operands (PSUM accumulation stays fp32); rel err ~5e-3
  - core <-> (branch, batch-quarter): each core runs all 25 patches of one
    branch on a 512-image quarter => 25 identical slots/core, perfect load
    balance (5050 matmuls/core vs 5252 on the worst v1 core)
  - per-slot weight array shipped as ONE 6.6MB bf16 DMA, double-buffered =>
    PE never waits on weights after the first slot
  - xs (the 9 input rows a patch-row needs) loaded once per patch-row i and
    shared by its 5 patches: 5 xs loads per core instead of 25
"""

import numpy as np
from contextlib import ExitStack

import ml_dtypes

import concourse.bass as bass
import concourse.mybir as mybir
import concourse.tile as tile
from concourse import bacc
from concourse.bass_utils import run_bass_kernel_spmd

# ---------------------------------------------------------------- constants
B = 2048
C_IN = 2
W1C = 16
W2C = 32
KK = 5
NP = 25
L = 10
OUT = 50
N_CORES = 8
BQ = 512               # images per slot (batch quarter; also the moving tile)

H_WIN = 9              # input rows needed per patch row (bicubic support)
XS_ROWS = H_WIN * 25 * C_IN   # 450 real rows
XS_PAD = 512                  # padded to 4 k-tiles
XUP_ROWS = 200                # (y'',x'',ci) per patch
H1_ROWS = 100 * W1C           # 1600
H2_ROWS = 100 * W2C           # 3200
OUT_ROWS = 100

f32 = mybir.dt.float32
bf16 = mybir.dt.bfloat16
BF16 = ml_dtypes.bfloat16


def _h0(i):
    return min(max(5 * i - 2, 0), 25 - H_WIN)


def bicubic_matrix(out_size, in_size):
    """Row-stochastic resize matrix identical to jax.image.resize bicubic."""
    scale = out_size / in_size
    u = (np.arange(out_size) + 0.5) / scale - 0.5
    s = np.abs(u[:, None] - np.arange(in_size)[None, :])
    A = -0.5
    w = np.where(
        s <= 1,
        (A + 2) * s**3 - (A + 3) * s**2 + 1,
        np.where(s < 2, A * s**3 - 5 * A * s**2 + 8 * A * s - 4 * A, 0.0),
    )
    w = w / w.sum(axis=1, keepdims=True)
    return w.astype(np.float64)


_R = bicubic_matrix(OUT, 25)           # [50, 25]
_R_NZ = np.abs(_R) > 1e-300


# ------------------------------------------------------- structural schedule
def _grids(krows, mrows, kc, mc, kh, mw):
    kl = kc * 128 + np.arange(kh)
    ml = mc * 128 + np.arange(mw)
    return kl[:, None], ml[None, :]


def _conv_struct(krows, mrows, kdiv, mdiv):
    """Sparsity pattern for T[(s'*kdiv+ci),(s*mdiv+co)] with 5x5 taps."""
    k = np.arange(krows)
    m = np.arange(mrows)
    sp, s = k // kdiv, m // mdiv
    yp, xp = sp // 10, sp % 10
    y, x = s // 10, s % 10
    return (np.abs(yp[:, None] - y[None, :]) <= 2) & (
        np.abs(xp[:, None] - x[None, :]) <= 2
    )


def _up_struct():
    """Union-over-patches sparsity of the upsample matrix [XS_PAD, XUP_ROWS]."""
    nz = np.zeros((XS_PAD, XUP_ROWS), dtype=bool)
    k = np.arange(XS_ROWS)
    hl, wv, ci = k // 50, (k % 50) // 2, k % 2
    m = np.arange(XUP_ROWS)
    spp, cip = m // 2, m % 2
    ypp, xpp = spp // 10, spp % 10
    for i in range(5):
        for j in range(5):
            lv = _R_NZ[10 * i + ypp, :][:, _h0(i) + hl]  # [M, K]
            rv = _R_NZ[10 * j + xpp, :][:, wv]
            nzij = (lv & rv).T & (ci[:, None] == cip[None, :])
            nz[:XS_ROWS] |= nzij
    return nz


def _blocks_of(nzmask, krows, mrows):
    """For each 128-col chunk, the list of 128-row chunks with any nonzero."""
    nkc = (krows + 127) // 128
    nmc = (mrows + 127) // 128
    out = []
    for mc in range(nmc):
        mw = min(128, mrows - mc * 128)
        kcs = []
        for kc in range(nkc):
            kh = min(128, krows - kc * 128)
            if nzmask[kc * 128 : kc * 128 + kh, mc * 128 : mc * 128 + mw].any():
                kcs.append((kc, kh))
        out.append((mw, kcs))
    return out


_SCHED = {
    "up": _blocks_of(_up_struct(), XS_PAD, XUP_ROWS),
    "c1": _blocks_of(_conv_struct(XUP_ROWS, H1_ROWS, 2, W1C), XUP_ROWS, H1_ROWS),
    "c2": _blocks_of(_conv_struct(H1_ROWS, H2_ROWS, W1C, W2C), H1_ROWS, H2_ROWS),
    "c3": _blocks_of(_conv_struct(H2_ROWS, OUT_ROWS, W2C, 1), H2_ROWS, OUT_ROWS),
}

# column layout of the per-unit weight-block array
_GROUPS = {}   # (layer, mc) -> (col_off, n_blocks)
_col = 0
for _lay in ("up", "c1", "c2", "c3"):
    for _mc, (_mw, _kcs) in enumerate(_SCHED[_lay]):
        _GROUPS[(_lay, _mc)] = (_col, len(_kcs))
        _col += len(_kcs) * 128
TOTCOLS = _col

_N_MC = {lay: len(_SCHED[lay]) for lay in _SCHED}
_BIAS_COLS = _N_MC["c1"] + _N_MC["c2"] + 1   # conv1, conv2, conv3 bias columns
_BC1, _BC2, _BC3 = 0, _N_MC["c1"], _N_MC["c1"] + _N_MC["c2"]

# per-patch up-stage schedule: the union layout has 8 blocks but a given
# patch only needs ~5.8 — skip its structurally-zero blocks (wb layout keeps
# the union column positions, so idx offsets are unchanged)
def _up_sched_for_patch(p):
    i, j = p // 5, p % 5
    k = np.arange(XS_ROWS)
    hl, wv, ci = k // 50, (k % 50) // 2, k % 2
    m = np.arange(XUP_ROWS)
    spp, cip = m // 2, m % 2
    ypp, xpp = spp // 10, spp % 10
    lv = _R_NZ[10 * i + ypp, :][:, _h0(i) + hl]
    rv = _R_NZ[10 * j + xpp, :][:, wv]
    nz = np.zeros((XS_PAD, XUP_ROWS), bool)
    nz[:XS_ROWS] = (lv & rv).T & (ci[:, None] == cip[None, :])
    out = []
    for mc, (mw, kcs) in enumerate(_SCHED["up"]):
        keep = []
        for idx, (kc, kh) in enumerate(kcs):
            if nz[kc * 128 : kc * 128 + kh, mc * 128 : mc * 128 + mw].any():
                keep.append((idx, kc, kh))
        out.append((mw, keep))
    return out


_UP_SCHED_P = [_up_sched_for_patch(p) for p in range(NP)]

# weight-DMA sub-transfers: [up+c1 | c2 in thirds | c3] so early layers
# of a slot can start before the whole 6.6MB slot array lands
_C2_OFF = _GROUPS[("c2", 0)][0]
_C3_OFF = _GROUPS[("c3", 0)][0]
_C2_CUT1 = _GROUPS[("c2", 8)][0]
_C2_CUT2 = _GROUPS[("c2", 17)][0]
_C1_OFF = _GROUPS[("c1", 0)][0]
_WSPLITS = [(0, _C1_OFF), (_C1_OFF, _C2_OFF),
            (_C2_OFF, _C2_CUT1), (_C2_CUT1, _C2_CUT2),
            (_C2_CUT2, _C3_OFF), (_C3_OFF, TOTCOLS)]

# 50 weight units ordered (branch, patch); core c runs units br=c//4
N_UNITS = 2 * NP


# ------------------------------------------------- host weight-block builder
def _conv_gather(krows, mrows, kdiv, mdiv, nci, kc, mc, kh, mw):
    """Gather indices+mask for a conv block: widx into w[co,ci,ky,kx] flat."""
    kl, ml = _grids(krows, mrows, kc, mc, kh, mw)
    sp, ci = kl // kdiv, kl % kdiv
    s, co = ml // mdiv, ml % mdiv
    yp, xp = sp // 10, sp % 10
    y, x = s // 10, s % 10
    ky = yp - y + 2
    kx = xp - x + 2
    valid = (ky >= 0) & (ky < 5) & (kx >= 0) & (kx < 5)
    ky = np.clip(ky, 0, 4)
    kx = np.clip(kx, 0, 4)
    widx = ((co * nci + ci) * 5 + ky) * 5 + kx
    return widx, valid


_CONV_GATHER_CACHE = {}


def _conv_gather_cached(lay, kc, mc, kh, mw):
    key = (lay, kc, mc)
    if key not in _CONV_GATHER_CACHE:
        if lay == "c1":
            g = _conv_gather(XUP_ROWS, H1_ROWS, 2, W1C, C_IN, kc, mc, kh, mw)
        elif lay == "c2":
            g = _conv_gather(H1_ROWS, H2_ROWS, W1C, W2C, W1C, kc, mc, kh, mw)
        else:
            g = _conv_gather(H2_ROWS, OUT_ROWS, W2C, 1, W2C, kc, mc, kh, mw)
        _CONV_GATHER_CACHE[key] = g
    return _CONV_GATHER_CACHE[key]


def _build_wblocks(w1f, w2f, w3f, ij):
    """Build [n_units, 128, TOTCOLS] float32 weight-block array.

    w1f [nu, 16*2*25], w2f [nu, 32*16*25], w3f [nu, 32*25]; ij [nu, 2]
    """
    nu = w1f.shape[0]
    wall = np.zeros((nu, 128, TOTCOLS), dtype=np.float32)

    # upsample blocks: value = R[10i+y'', h0(i)+h_loc] * R[10j+x'', w] * (ci==ci')
    k = np.arange(XS_PAD)
    hl, wv, ci = k // 50, (k % 50) // 2, k % 2
    hl = np.where(k < XS_ROWS, hl, 0)
    m = np.arange(XUP_ROWS)
    spp, cip = m // 2, m % 2
    ypp, xpp = spp // 10, spp % 10
    iu = ij[:, 0]
    ju = ij[:, 1]
    h0u = np.minimum(np.maximum(5 * iu - 2, 0), 25 - H_WIN)
    # left[u, m, k] = R[10*iu+ypp[m], h0u+hl[k]]  (zero beyond real rows)
    left = _R[(10 * iu[:, None, None] + ypp[None, :, None]),
              (h0u[:, None, None] + hl[None, None, :])]
    left *= (np.arange(XS_PAD)[None, None, :] < XS_ROWS)
    right = _R[(10 * ju[:, None, None] + xpp[None, :, None]),
               wv[None, None, :]]
    same = (ci[None, :] == cip[:, None])[None, :, :]          # [1, M, K]
    upmat = (left * right * same).transpose(0, 2, 1).astype(np.float32)  # [nu,K,M]
    for mc, (mw, kcs) in enumerate(_SCHED["up"]):
        off, _n = _GROUPS[("up", mc)]
        for idx, (kc, kh) in enumerate(kcs):
            wall[:, :kh, off + idx * 128 : off + idx * 128 + mw] = upmat[
                :, kc * 128 : kc * 128 + kh, mc * 128 : mc * 128 + mw
            ]

    for lay, wf in (("c1", w1f), ("c2", w2f), ("c3", w3f)):
        for mc, (mw, kcs) in enumerate(_SCHED[lay]):
            off, _n = _GROUPS[(lay, mc)]
            for idx, (kc, kh) in enumerate(kcs):
                widx, valid = _conv_gather_cached(lay, kc, mc, kh, mw)
                blk = wf[:, widx.reshape(-1)].reshape(nu, kh, mw)
                blk = blk * valid[None, :, :]
                wall[:, :kh, off + idx * 128 : off + idx * 128 + mw] = blk
    return wall


def _relu_bias(eng, out_ap, ps_ap, bias_ap):
    """out = max(ps + bias, 0) — ACT engine via activation, DVE/Pool via
    fused tensor_scalar."""
    import concourse.mybir as _mb
    if hasattr(eng, "activation"):
        eng.activation(out_ap, ps_ap, _mb.ActivationFunctionType.Relu,
                       bias=bias_ap)
    else:
        eng.tensor_scalar(out_ap, ps_ap, bias_ap, 0.0,
                          _mb.AluOpType.add, _mb.AluOpType.max)


# --------------------------------------------------------- device program
_NC_CACHE = None


def _build_nc():
    nc = bacc.Bacc("TRN2", target_bir_lowering=False, debug=False,
                   num_devices=N_CORES)
    xs_d = nc.dram_tensor("xs", [5, XS_PAD, BQ], bf16, kind="ExternalInput").ap()
    wb_d = nc.dram_tensor("wb", [NP, 128, TOTCOLS], bf16,
                          kind="ExternalInput").ap()
    bias_d = nc.dram_tensor("bias", [NP, 128, _BIAS_COLS], f32,
                            kind="ExternalInput").ap()
    y_d = nc.dram_tensor("y", [NP, OUT_ROWS, BQ], f32, kind="ExternalOutput").ap()

    AF = mybir.ActivationFunctionType
    mw3, kcs3 = _SCHED["c3"][0]
    kh3_map = dict(kcs3)
    off3, _n3 = _GROUPS[("c3", 0)]
    n_c2 = _N_MC["c2"]

    with tile.TileContext(nc) as tc, ExitStack() as ctx:
        xs_pool = ctx.enter_context(tc.tile_pool(name="xs", bufs=5))
        xup_pool = ctx.enter_context(tc.tile_pool(name="xup", bufs=2))
        h1_pool = ctx.enter_context(tc.tile_pool(name="h1", bufs=1))
        h2_pool = ctx.enter_context(tc.tile_pool(name="h2", bufs=6))
        w_pool = ctx.enter_context(tc.tile_pool(name="w", bufs=2))
        bias_pool = ctx.enter_context(tc.tile_pool(name="bias", bufs=2))
        out_pool = ctx.enter_context(tc.tile_pool(name="out", bufs=2))
        ps_pool = ctx.enter_context(tc.tile_pool(name="ps", bufs=6, space="PSUM"))
        ps3_pool = ctx.enter_context(tc.tile_pool(name="ps3", bufs=2, space="PSUM"))

        for i in range(5):
            # xs for this patch-row, shared by its 5 patches
            xs_t = []
            for t in range(XS_PAD // 128):
                xt = xs_pool.tile([128, BQ], bf16, tag=f"xs{t}")
                nc.sync.dma_start(out=xt[:], in_=xs_d[i, 128 * t : 128 * (t + 1), :])
                xs_t.append(xt)

            for j in range(5):
                u = 5 * i + j
                # whole-slot weight array, one transfer, double-buffered
                wt = w_pool.tile([128, TOTCOLS], bf16, tag="w")
                for c0, c1 in _WSPLITS:
                    nc.gpsimd.dma_start(out=wt[:, c0:c1], in_=wb_d[u, :, c0:c1])
                bias_t = bias_pool.tile([128, _BIAS_COLS], f32, tag="bias")
                nc.sync.dma_start(out=bias_t[:], in_=bias_d[u])

                def mm_group(lay, mc, mw, kcs, src_tiles, ps):
                    off, _n = _GROUPS[(lay, mc)]
                    for idx, (kc, kh) in enumerate(kcs):
                        nc.tensor.matmul(
                            ps[:mw, :],
                            wt[:kh, off + idx * 128 : off + idx * 128 + mw],
                            src_tiles[kc][:kh, :],
                            start=(idx == 0),
                            stop=(idx == len(kcs) - 1),
                        )

                # ---- upsample -> xup tiles (patch-specific block subset)
                xup_t = []
                for mc, (mw, keep) in enumerate(_UP_SCHED_P[u]):
                    off, _n = _GROUPS[("up", mc)]
                    ps = ps_pool.tile([128, BQ], f32, tag="ps")
                    for n_i, (idx, kc, kh) in enumerate(keep):
                        nc.tensor.matmul(
                            ps[:mw, :],
                            wt[:kh, off + idx * 128 : off + idx * 128 + mw],
                            xs_t[kc][:kh, :],
                            start=(n_i == 0),
                            stop=(n_i == len(keep) - 1),
                        )
                    xt2 = xup_pool.tile([128, BQ], bf16, tag=f"xup{mc}")
                    nc.vector.tensor_copy(xt2[:mw, :], ps[:mw, :])
                    xup_t.append(xt2)

                # ---- conv1 -> h1 tiles (relu + bias)
                h1_t = []
                for mc, (mw, kcs) in enumerate(_SCHED["c1"]):
                    ps = ps_pool.tile([128, BQ], f32, tag="ps")
                    mm_group("c1", mc, mw, kcs, xup_t, ps)
                    ht = h1_pool.tile([128, BQ], bf16, tag=f"h1_{mc}")
                    _relu_bias(nc.scalar if mc % 2 == 0 else nc.vector,
                               ht[:mw, :], ps[:mw, :],
                               bias_t[:mw, _BC1 + mc : _BC1 + mc + 1])
                    h1_t.append(ht)

                # ---- conv2 + interleaved conv3 accumulation
                # conv3's matmul for group mc is emitted one c2-group late so
                # the PE never waits on the h2 activation latency
                ps3 = ps3_pool.tile([128, BQ], f32, tag="ps3")

                def c3_mm(mc, h2t):
                    nc.tensor.matmul(
                        ps3[:mw3, :],
                        wt[: kh3_map[mc], off3 + mc * 128 : off3 + mc * 128 + mw3],
                        h2t[: kh3_map[mc], :],
                        start=(mc == 0),
                        stop=(mc == n_c2 - 1),
                        skip_group_check=True,
                    )

                pending = None   # (mc, h2t) awaiting its conv3 matmul
                for mc, (mw, kcs) in enumerate(_SCHED["c2"]):
                    ps = ps_pool.tile([128, BQ], f32, tag="ps")
                    mm_group("c2", mc, mw, kcs, h1_t, ps)
                    if pending is not None:
                        c3_mm(*pending)
                    h2t = h2_pool.tile([128, BQ], bf16, tag="h2")
                    _relu_bias(nc.scalar if mc % 2 == 0 else nc.vector,
                               h2t[:mw, :], ps[:mw, :],
                               bias_t[:mw, _BC2 + mc : _BC2 + mc + 1])
                    pending = (mc, h2t)
                c3_mm(*pending)

                # ---- conv3 bias + store
                ot = out_pool.tile([128, BQ], f32, tag="out")
                nc.scalar.activation(
                    ot[:mw3, :], ps3[:mw3, :], AF.Identity,
                    bias=bias_t[:mw3, _BC3 : _BC3 + 1],
                )
                nc.sync.dma_start(out=y_d[u], in_=ot[:OUT_ROWS, :])

    nc.compile()
    return nc


# ----------------------------------------------------------- host pipeline
LAST_RESULTS = None


def _unit_weight_flats(inputs):
    """Per-unit flattened conv weights/biases, unit order = (branch, patch)."""
    w1 = np.stack([np.asarray(inputs["phi_w1"]), np.asarray(inputs["p_w1"])], 0)
    w2 = np.stack([np.asarray(inputs["phi_w2"]), np.asarray(inputs["p_w2"])], 0)
    w3 = np.stack([np.asarray(inputs["phi_w3"]), np.asarray(inputs["p_w3"])], 0)
    b1 = np.stack([np.asarray(inputs["phi_b1"]), np.asarray(inputs["p_b1"])], 0)
    b2 = np.stack([np.asarray(inputs["phi_b2"]), np.asarray(inputs["p_b2"])], 0)
    b3 = np.stack([np.asarray(inputs["phi_b3"]), np.asarray(inputs["p_b3"])], 0)
    # [br, p, ...] -> [br*25+p, flat]
    w1f = w1.reshape(N_UNITS, -1).astype(np.float32)
    w2f = w2.reshape(N_UNITS, -1).astype(np.float32)
    w3f = w3.reshape(N_UNITS, -1).astype(np.float32)
    b1u = b1.reshape(N_UNITS, W1C).astype(np.float32)
    b2u = b2.reshape(N_UNITS, W2C).astype(np.float32)
    b3u = b3.reshape(N_UNITS).astype(np.float32)
    p = np.tile(np.arange(NP), 2)
    ij = np.stack([p // 5, p % 5], 1)
    return w1f, w2f, w3f, b1u, b2u, b3u, ij


def _build_host_inputs(inputs):
    x = np.asarray(inputs["x"], dtype=np.float32)
    w1f, w2f, w3f, b1u, b2u, b3u, ij = _unit_weight_flats(inputs)
    wall = _build_wblocks(w1f, w2f, w3f, ij).astype(BF16)  # [50, 128, TOTCOLS]

    # biases [50, 128, BIAS_COLS]
    ball = np.zeros((N_UNITS, 128, _BIAS_COLS), np.float32)
    for mc, (mw, _k) in enumerate(_SCHED["c1"]):
        ml = mc * 128 + np.arange(mw)
        ball[:, :mw, _BC1 + mc] = b1u[:, ml % W1C]
    for mc, (mw, _k) in enumerate(_SCHED["c2"]):
        ml = mc * 128 + np.arange(mw)
        ball[:, :mw, _BC2 + mc] = b2u[:, ml % W2C]
    ball[:, :OUT_ROWS, _BC3] = b3u[:, None]

    # x slices per patch row i: [5, XS_PAD, B]
    xt = np.ascontiguousarray(x.transpose(2, 3, 1, 0))  # [h, w, ci, b]
    xs_all = np.zeros((5, XS_PAD, B), np.float32)
    for i in range(5):
        h0 = _h0(i)
        xs_all[i, :XS_ROWS] = xt[h0 : h0 + H_WIN].reshape(XS_ROWS, B)
    xs_all = xs_all.astype(BF16)

    in_maps = []
    for c in range(N_CORES):
        br, q = c // 4, c % 4
        in_maps.append({
            "xs": np.ascontiguousarray(xs_all[:, :, q * BQ : (q + 1) * BQ]),
            "wb": wall[br * NP : (br + 1) * NP],
            "bias": ball[br * NP : (br + 1) * NP],
        })
    return in_maps


def _assemble(results):
    out = np.zeros((B, 2, OUT, OUT), np.float32)
    for c in range(N_CORES):
        br, q = c // 4, c % 4
        y = results[c]["y"]  # [25, 100, BQ]
        for p in range(NP):
            i, j = p // 5, p % 5
            blk = y[p].reshape(L, L, BQ).transpose(2, 0, 1)
            out[q * BQ : (q + 1) * BQ, br,
                10 * i : 10 * i + L, 10 * j : 10 * j + L] = blk
    return out


def kernel(**inputs):
    global _NC_CACHE, LAST_RESULTS
    in_maps = _build_host_inputs(inputs)
    if _NC_CACHE is None:
        _NC_CACHE = _build_nc()
    res = run_bass_kernel_spmd(_NC_CACHE, in_maps, list(range(N_CORES)))
    LAST_RESULTS = res
    return _assemble(res.results)


# ------------------------------------------------- numpy emulation (debug)
def emulate(**inputs):
    """Pure-numpy emulation of the exact device dataflow (for debugging)."""
    in_maps = _build_host_inputs(inputs)
    results = []
    for c in range(N_CORES):
        m = in_maps[c]
        wbf = np.asarray(m["wb"], dtype=np.float32)
        xsf = np.asarray(m["xs"], dtype=np.float32)
        y = np.zeros((NP, OUT_ROWS, BQ), np.float32)
        for u in range(NP):
            acts = {"xs": xsf[u // 5]}
            srcs = {"up": "xs", "c1": "up", "c2": "c1", "c3": "c2"}
            rows = {"up": XUP_ROWS, "c1": H1_ROWS, "c2": H2_ROWS, "c3": OUT_ROWS}
            for lay in ("up", "c1", "c2", "c3"):
                src = acts[srcs[lay]]
                dst = np.zeros((rows[lay], BQ), np.float32)
                for mc, (mw, kcs) in enumerate(_SCHED[lay]):
                    off, _n = _GROUPS[(lay, mc)]
                    pacc = np.zeros((mw, BQ), np.float32)
                    for idx, (kc, kh) in enumerate(kcs):
                        blk = wbf[u][:kh, off + idx * 128 : off + idx * 128 + mw]
                        pacc += blk.T @ src[kc * 128 : kc * 128 + kh]
                    dst[mc * 128 : mc * 128 + mw] = pacc
                if lay == "c1":
                    dst = np.maximum(
                        dst + m["bias"][u][
                            np.arange(H1_ROWS) % 128, _BC1 + np.arange(H1_ROWS) // 128
                        ][:, None], 0)
                elif lay == "c2":
                    dst = np.maximum(
                        dst + m["bias"][u][
                            np.arange(H2_ROWS) % 128, _BC2 + np.arange(H2_ROWS) // 128
                        ][:, None], 0)
                elif lay == "c3":
                    dst = dst + m["bias"][u][:OUT_ROWS, _BC3][:, None]
                    acts[lay] = dst
                    continue
                # device stores inter-layer activations in bf16
                acts[lay] = q_bf16(dst)
            y[u] = acts["c3"]
        results.append({"y": y})
    return _assemble(results)


def q_bf16(a):
    return a.astype(BF16).astype(np.float32)
